# revision 9
# baseline (speedup 1.0000x reference)
"""Causal multi-head attention (B=4, H=16, S=2048, D=64) on 8 TRN2 NeuronCores.

Sharding: B*H = 64 heads, 8 heads per core (data/head parallel, no comms).

Per-core pipeline (per head), v2 — exp spread over three engines:
  - DMA Q,K,V [2048,64] f32 -> SBUF, cast to bf16 (DVE)
  - Q,K to d-major via DMA XBAR pair-transposes ([128 s, 2x64 d] -> [128,128],
    even tile's d rows on partitions 0:64, odd tile's on 64:128), plus one
    bulk partition-shift DMA per operand moving odd halves to base 0
    (matmul requires equal base partitions); K=64-contraction matmuls
  - QK^T strips E^T[k, q] packed into [128, <=1536] f32 PSUM pieces
    (12/head), with even/odd q-tile families as strided-out matmuls
  - exp: each piece goes to ONE of: ScalarE ACT Exp, DVE Schraudolph
    (bits = int16(e*scale*184.66 + beta) bitcast to bf16), or GpSimd
    Schraudolph -- balancing the three engines; strips land contiguously
    in one at_all [128, 17408] bf16 tile per head
  - causal zeroing of diagonal tiles via trimask multiply (gpsimd/DVE)
  - A@V with ones-column appended to V: o4 groups of 4 q-tiles in PSUM,
    normalize with DVE reciprocal + scale, stream out per group
"""

import os
import sys

try:
    import concourse.bass as bass  # noqa: F401
except ImportError:
    sys.path.insert(0, "/opt/trn_rl_repo")
    import concourse.bass as bass  # noqa: F401

import numpy as np

import concourse.mybir as mybir
import concourse.tile as tile
from concourse import bacc
from concourse.bass_utils import run_bass_kernel_spmd

B, H, S, D = 4, 16, 2048, 64
N_CORES = 8
HEADS = B * H
HPC = HEADS // N_CORES  # heads per core
P = 128
ST = S // P  # 16 s-tiles per head
NP = ST // 2  # 8 pairs

F32 = mybir.dt.float32
BF16 = mybir.dt.bfloat16
I16 = mybir.dt.int16

SCALE = 1.0 / float(np.sqrt(D))
LN2 = float(np.log(2.0))
ALPHA = 128.0 / LN2 * SCALE          # Schraudolph slope on raw scores
BETA = 127.0 * 128.0 - 7.33          # exponent bias, centered for min RMS err

PIECE = int(os.environ.get("K_PIECE", "1536"))   # PSUM piece columns
AVLAG = int(os.environ.get("K_AVLAG", "1"))      # A@V lag in pieces
# per-piece exp engine pattern (A=ACT, D=DVE Schraudolph, G=GpSimd Schraudolph)
PAT = os.environ.get("K_PAT", "AADAADADADAA")
MASKENG = os.environ.get("K_MASKENG", "gp")      # gp | dve
NORMENG = os.environ.get("K_NORMENG", "dve")     # dve | gp
NSPLIT0 = int(os.environ.get("K_NSPLIT0", "2"))

WJ = [S - P * j for j in range(ST)]              # strip widths
AT_OFF = [0] * (ST + 1)
for j in range(ST):
    AT_OFF[j + 1] = AT_OFF[j] + WJ[j]
AT_TOT = AT_OFF[ST]                              # 17408


def plan_pieces():
    """Greedy pack strips (in order) into PIECE-col pieces.

    Within each fragment the q-tiles are laid out evens-first-then-odds so
    each matmul family writes a CONTIGUOUS psum range (psum bank rule:
    one matmul may not cross a 512-f32 bank boundary).  POS[(j, t)] gives
    the at_all column where strip j's q-tile t lands.

    Returns (pieces, pos): pieces = [(at0, cols, frags, done, diag)] with
    frags = [(j, base_at, [even tiles], [odd tiles])]."""
    pieces = []
    pos = {}
    at0 = 0
    j, q = 0, 0  # strip cursor: strip j, local col q (0..WJ[j])
    while j < ST:
        cols = 0
        frags = []
        done = []
        diag = []
        while j < ST and cols < PIECE:
            take = min(WJ[j] - q, PIECE - cols)
            if q == 0:
                diag.append(j)
            t0, t1 = j + q // P, j + (q + take) // P
            evens = [t for t in range(t0, t1) if t % 2 == 0]
            odds = [t for t in range(t0, t1) if t % 2 == 1]
            base = at0 + cols
            for i, t in enumerate(evens + odds):
                pos[(j, t)] = base + P * i
            frags.append((j, base, evens, odds))
            q += take
            cols += take
            if q == WJ[j]:
                done.append(j)
                j += 1
                q = 0
        pieces.append((at0, cols, frags, done, diag))
        at0 += cols
    return pieces, pos


PIECES, POS = plan_pieces()
NPC = len(PIECES)
# strip jq ready after piece index r(jq)
READY = [0] * ST
for pi, (_, _, _, done, _) in enumerate(PIECES):
    for j in done:
        READY[j] = pi


def build_nc(heads_per_core=HPC):
    nc = bacc.Bacc("TRN2", target_bir_lowering=False, debug=False,
                   num_devices=N_CORES)
    q_d = nc.dram_tensor("Q", [heads_per_core, S, D], F32, kind="ExternalInput")
    k_d = nc.dram_tensor("K", [heads_per_core, S, D], F32, kind="ExternalInput")
    v_d = nc.dram_tensor("V", [heads_per_core, S, D], F32, kind="ExternalInput")
    o_d = nc.dram_tensor("out", [heads_per_core, S, D], F32, kind="ExternalOutput")

    with tile.TileContext(nc) as tc:
        with (
            tc.tile_pool(name="const", bufs=1) as const,
            tc.tile_pool(name="stage", bufs=2) as stage,
            tc.tile_pool(name="bfp", bufs=2) as bfp,
            tc.tile_pool(name="tp", bufs=2) as tpool,
            tc.tile_pool(name="atp", bufs=2) as atp,
            tc.tile_pool(name="osb", bufs=2) as osbp,
            tc.tile_pool(name="small", bufs=8) as small,
            tc.tile_pool(name="ps", bufs=1, space="PSUM") as ps,
        ):
            # upper-triangular (incl. diagonal) ones: keep q >= k
            trimask = const.tile([P, P], BF16, tag="trimask")
            nc.gpsimd.memset(trimask, 1.0)
            nc.gpsimd.affine_select(
                out=trimask, in_=trimask,
                compare_op=mybir.AluOpType.is_ge,
                fill=0.0, base=0,
                pattern=[[1, P]], channel_multiplier=-1,
            )

            def emit_prep(h, nsplit=1):
                """Load + cast + DMA-transpose head h's operands."""
                q_raw = stage.tile([P, ST, D], F32, tag="qraw")
                k_raw = stage.tile([P, ST, D], F32, tag="kraw")
                v_raw = stage.tile([P, ST, D], F32, tag="vraw")
                q_bf = bfp.tile([P, ST, D], BF16, tag="qbf")
                k_bf = bfp.tile([P, ST, D], BF16, tag="kbf")
                qTp = tpool.tile([P, NP, P], BF16, tag="qTp")
                kTp = tpool.tile([P, NP, P], BF16, tag="kTp")
                qTo = tpool.tile([64, NP, P], BF16, tag="qTo")
                kTo = tpool.tile([64, NP, P], BF16, tag="kTo")
                splits = [(ST * i // nsplit, ST * (i + 1) // nsplit)
                          for i in range(nsplit)]
                for s0, s1 in splits:
                    for (raw, d_) in ((q_raw, q_d), (k_raw, k_d)):
                        nc.sync.dma_start(
                            out=raw[:, s0:s1, :],
                            in_=d_[h].rearrange("(b p) d -> p b d", p=P)[:, s0:s1, :])
                for si, (s0, s1) in enumerate(splits):
                    if si == 1 or nsplit == 1:
                        # defer V: only Q/K gate the QK^T ramp
                        nc.sync.dma_start(
                            out=v_raw, in_=v_d[h].rearrange("(b p) d -> p b d", p=P))
                    p0, p1 = s0 // 2, s1 // 2
                    for (raw, bf_, t_, o_) in (
                        (q_raw, q_bf, qTp, qTo),
                        (k_raw, k_bf, kTp, kTo),
                    ):
                        nc.vector.tensor_copy(bf_[:, s0:s1, :], raw[:, s0:s1, :])
                        for pr in range(p0, p1):
                            nc.sync.dma_start_transpose(
                                t_[:, pr, :],
                                bf_[:, 2 * pr:2 * pr + 2, :].rearrange(
                                    "p a d -> p (a d)"),
                            )
                        # odd halves to base partition 0 (one bulk shift)
                        nc.sync.dma_start(out=o_[:, p0:p1, :],
                                          in_=t_[64:P, p0:p1, :])
                v_aug = bfp.tile([P, ST, D + 1], BF16, tag="vaug")
                nc.vector.tensor_copy(v_aug[:, :, 0:D], v_raw)
                nc.vector.memset(v_aug[:, :, D:D + 1], 1.0)
                at_all = atp.tile([P, AT_TOT], BF16, tag="at")
                return {
                    "qTp": qTp, "qTo": qTo, "kTp": kTp, "kTo": kTo,
                    "v_aug": v_aug, "at": at_all,
                    "o_sb": osbp.tile([P, ST, D], F32, tag="osb",
                                      name=f"osb{h}"),
                }

            state = {}

            def emit_piece(h, pi):
                """QK^T matmuls for piece pi of head h, then exp, then masks."""
                st = state[h]
                at0, cols, frags, done, diag = PIECES[pi]
                et = ps.tile([P, PIECE], F32, tag="et", bufs=2, name="et")
                for (j, fbase, evens, odds) in frags:
                    # k-tile j: lhsT at base 0
                    if j % 2 == 0:
                        lhsT = st["kTp"][0:64, j // 2, :]
                    else:
                        lhsT = st["kTo"][:, j // 2, :]
                    base = fbase - at0                   # piece-local col
                    for g, ts in ((0, evens), (1, odds)):
                        if not ts:
                            continue
                        src = st["qTp"] if g == 0 else st["qTo"]
                        if g == 1:
                            base += P * len(evens)
                        i = 0
                        while i < len(ts):
                            # chunk ends at psum bank boundary (512 f32)
                            c = base + P * i
                            room = (512 - c % 512) // P
                            n = min(4, len(ts) - i, max(room, 1))
                            pq0 = ts[i] // 2
                            nc.tensor.matmul(
                                et[:, c:c + P * n],
                                lhsT=lhsT,
                                rhs=(src[0:64, pq0:pq0 + n, :] if g == 0
                                     else src[:, pq0:pq0 + n, :]),
                                start=True, stop=True,
                            )
                            i += n
                dst = st["at"][:, at0:at0 + cols]
                eng = PAT[pi % len(PAT)]
                if eng == "A":
                    nc.scalar.activation(
                        dst, et[:, 0:cols],
                        mybir.ActivationFunctionType.Exp,
                        scale=SCALE,
                    )
                else:
                    e_ = nc.vector if eng == "D" else nc.gpsimd
                    e_.tensor_scalar(
                        out=dst.bitcast(I16),
                        in0=et[:, 0:cols],
                        scalar1=ALPHA,
                        scalar2=BETA,
                        op0=mybir.AluOpType.mult,
                        op1=mybir.AluOpType.add,
                    )
                for j in diag:
                    m_ = nc.gpsimd if MASKENG == "gp" else nc.vector
                    dg = st["at"][:, POS[(j, j)]:POS[(j, j)] + P]
                    m_.tensor_tensor(out=dg, in0=dg, in1=trimask,
                                     op=mybir.AluOpType.mult)

            def emit_av(h, jq):
                """A@V for q-tile jq of head h; groups of 4 share a PSUM bank."""
                st = state[h]
                at_all, v_aug, o_sb = st["at"], st["v_aug"], st["o_sb"]
                if jq % 4 == 0:
                    st["o4"] = ps.tile([P, 4, D + 1], F32, tag="o",
                                       bufs=2, name="o4")
                o4 = st["o4"]
                for k in range(jq + 1):
                    c = POS[(k, jq)]
                    nc.tensor.matmul(
                        o4[:, jq % 4, :],
                        lhsT=at_all[:, c:c + P],
                        rhs=v_aug[:, k, :],
                        start=(k == 0), stop=(k == jq),
                    )
                if jq % 4 == 3:
                    recip4 = small.tile([P, 4], F32, tag="recip")
                    nc.vector.reciprocal(
                        recip4,
                        o4[:, :, D:D + 1].rearrange("p a b -> p (a b)"),
                    )
                    rb = bass.AP(tensor=recip4.tensor, offset=recip4.offset,
                                 ap=[recip4.ap[0], recip4.ap[1], [0, D]])
                    n_ = nc.vector if NORMENG == "dve" else nc.gpsimd
                    n_.tensor_tensor(
                        out=o_sb[:, jq - 3:jq + 1, :],
                        in0=o4[:, :, 0:D], in1=rb,
                        op=mybir.AluOpType.mult,
                    )
                    nc.sync.dma_start(
                        out=o_d[h].rearrange("(b p) d -> p b d", p=P)
                                  [:, jq - 3:jq + 1, :],
                        in_=o_sb[:, jq - 3:jq + 1, :],
                    )
                if jq == ST - 1:
                    del state[h]

            # flattened pipeline over (head, piece); A@V trails by AVLAG pieces
            av_tasks = [(h, jq) for h in range(heads_per_core)
                        for jq in range(ST)]
            av_ready_gpi = {}
            for h in range(heads_per_core):
                for jq in range(ST):
                    av_ready_gpi[(h, jq)] = h * NPC + READY[jq]
            state[0] = emit_prep(0, nsplit=NSPLIT0)
            av_next = 0
            for h in range(heads_per_core):
                for pi in range(NPC):
                    gpi = h * NPC + pi
                    emit_piece(h, pi)
                    if pi == 7 and h + 1 < heads_per_core:
                        state[h + 1] = emit_prep(h + 1)
                    while (av_next < len(av_tasks)
                           and av_ready_gpi[av_tasks[av_next]] + AVLAG <= gpi):
                        emit_av(*av_tasks[av_next])
                        av_next += 1
            while av_next < len(av_tasks):
                emit_av(*av_tasks[av_next])
                av_next += 1

    nc.compile()
    return nc


_NC_CACHE = {}


def _get_nc(heads_per_core=HPC):
    if heads_per_core not in _NC_CACHE:
        _NC_CACHE[heads_per_core] = build_nc(heads_per_core)
    return _NC_CACHE[heads_per_core]


def run_sharded(Q, K, V, heads_per_core=HPC, **run_kwargs):
    """Q, K, V: [HEADS-or-subset, S, D] f32 flattened over (B, H)."""
    nc = _get_nc(heads_per_core)
    n = heads_per_core
    in_maps = [
        {
            "Q": np.ascontiguousarray(Q[i * n:(i + 1) * n]),
            "K": np.ascontiguousarray(K[i * n:(i + 1) * n]),
            "V": np.ascontiguousarray(V[i * n:(i + 1) * n]),
        }
        for i in range(N_CORES)
    ]
    last_err = None
    for attempt in range(3):
        try:
            res = run_bass_kernel_spmd(nc, in_maps,
                                       core_ids=list(range(N_CORES)),
                                       **run_kwargs)
            out = np.concatenate(
                [np.asarray(res.results[i]["out"]) for i in range(N_CORES)],
                axis=0)
            return out, res
        except Exception as e:  # transient NRT_EXEC_UNIT_UNRECOVERABLE etc.
            last_err = e
            import time
            time.sleep(2.0)
    raise last_err


def kernel(Q, K, V, mask=None):
    Q = np.asarray(Q, dtype=np.float32).reshape(HEADS, S, D)
    K = np.asarray(K, dtype=np.float32).reshape(HEADS, S, D)
    V = np.asarray(V, dtype=np.float32).reshape(HEADS, S, D)
    out, _ = run_sharded(Q, K, V)
    return out.reshape(B, H, S, D)


# revision 12
# speedup vs baseline: 1.4532x; 1.4532x over previous
"""Causal multi-head attention (B=4, H=16, S=2048, D=64) on 8 TRN2 NeuronCores.

Sharding: B*H = 64 heads, 8 heads per core (data/head parallel, no comms).

Per-core pipeline (per head), v3 — exp spread over three engines:
  - DMA Q,K,V [2048,64] f32 -> SBUF
  - Q,K to d-major via PE pair-transposes reading f32 raws directly
    ([128 s, 2x64 d] -> [128,128] f32 PSUM; no separate bf16 cast pass),
    flattened to [128, 2048] bf16 with top 64 partitions zeroed (K=128
    contraction pad): even s-tiles via DVE copy-cast, odd s-tiles via DVE
    copy-cast + partition-shift DMA
  - QK^T strips E^T[k, q] packed into [128, <=1536] f32 PSUM pieces
    (12/head), matmul chunks <=512 cols split at PSUM bank boundaries
  - exp: each piece goes to ONE of: ScalarE ACT Exp, DVE Schraudolph
    (bits = int16(e*ALPHA + BETA) bitcast to bf16), or GpSimd Schraudolph
    after a PSUM->SBUF DMA bounce -- balancing the three engines; strips
    land contiguously in one at_all [128, 17408] bf16 tile per head
  - causal zeroing of diagonal tiles via trimask multiply (gpsimd/DVE)
  - A@V with ones-column appended to V: o4 groups of 4 q-tiles in PSUM,
    normalize with DVE reciprocal + scale, stream out per group
"""

import os
import sys

try:
    import concourse.bass as bass  # noqa: F401
except ImportError:
    sys.path.insert(0, "/opt/trn_rl_repo")
    import concourse.bass as bass  # noqa: F401

import numpy as np

import concourse.mybir as mybir
import concourse.tile as tile
from concourse import bacc
from concourse.bass_utils import run_bass_kernel_spmd
from concourse.masks import make_identity

B, H, S, D = 4, 16, 2048, 64
N_CORES = 8
HEADS = B * H
HPC = HEADS // N_CORES  # heads per core
P = 128
ST = S // P  # 16 s-tiles per head

F32 = mybir.dt.float32
BF16 = mybir.dt.bfloat16
I16 = mybir.dt.int16

SCALE = 1.0 / float(np.sqrt(D))
LN2 = float(np.log(2.0))
ALPHA = 128.0 / LN2 * SCALE          # Schraudolph slope on raw scores
BETA = 127.0 * 128.0 - 7.33          # exponent bias, centered for min RMS err

PIECE = int(os.environ.get("K_PIECE", "1536"))   # PSUM piece columns
AVLAG = int(os.environ.get("K_AVLAG", "1"))      # A@V lag in pieces
# per-piece exp engine pattern (A=ACT, D=DVE, G=GpSimd via SBUF bounce)
PAT = os.environ.get("K_PAT", "AADAADAADAADAADAADAADAAA")
MASKENG = os.environ.get("K_MASKENG", "gp")      # gp | dve
NORMENG = os.environ.get("K_NORMENG", "dve")     # dve | gp
NSPLIT0 = int(os.environ.get("K_NSPLIT0", "2"))

WJ = [S - P * j for j in range(ST)]              # strip widths
AT_OFF = [0] * (ST + 1)
for j in range(ST):
    AT_OFF[j + 1] = AT_OFF[j] + WJ[j]
AT_TOT = AT_OFF[ST]                              # 17408


def plan_pieces():
    """Greedy pack strips (in order) into PIECE-col pieces.
    Each piece: (at0, cols, frags, done, diag), frags = [(j, qa, qb)] in
    absolute q coords (128-aligned)."""
    pieces = []
    at0 = 0
    j, q = 0, 0
    while j < ST:
        cols = 0
        frags = []
        done = []
        diag = []
        while j < ST and cols < PIECE:
            take = min(WJ[j] - q, PIECE - cols)
            if q == 0:
                diag.append(j)
            frags.append((j, P * j + q, P * j + q + take))
            q += take
            cols += take
            if q == WJ[j]:
                done.append(j)
                j += 1
                q = 0
        pieces.append((at0, cols, frags, done, diag))
        at0 += cols
    return pieces


PIECES = plan_pieces()
NPC = len(PIECES)
READY = [0] * ST
for pi, (_, _, _, done, _) in enumerate(PIECES):
    for j in done:
        READY[j] = pi


def build_nc(heads_per_core=HPC):
    nc = bacc.Bacc("TRN2", target_bir_lowering=False, debug=False,
                   num_devices=N_CORES)
    q_d = nc.dram_tensor("Q", [heads_per_core, S, D], F32, kind="ExternalInput")
    k_d = nc.dram_tensor("K", [heads_per_core, S, D], F32, kind="ExternalInput")
    v_d = nc.dram_tensor("V", [heads_per_core, S, D], F32, kind="ExternalInput")
    o_d = nc.dram_tensor("out", [heads_per_core, S, D], F32, kind="ExternalOutput")

    with tile.TileContext(nc) as tc:
        with (
            tc.tile_pool(name="const", bufs=1) as const,
            tc.tile_pool(name="stage", bufs=2) as stage,
            tc.tile_pool(name="tp", bufs=2) as tpool,
            tc.tile_pool(name="atp", bufs=2) as atp,
            tc.tile_pool(name="osb", bufs=2) as osbp,
            tc.tile_pool(name="small", bufs=8) as small,
            tc.tile_pool(name="ps", bufs=1, space="PSUM") as ps,
        ):
            identity = const.tile([P, P], F32, tag="ident")
            make_identity(nc, identity)
            # upper-triangular (incl. diagonal) ones: keep q >= k
            trimask = const.tile([P, P], BF16, tag="trimask")
            nc.gpsimd.memset(trimask, 1.0)
            nc.gpsimd.affine_select(
                out=trimask, in_=trimask,
                compare_op=mybir.AluOpType.is_ge,
                fill=0.0, base=0,
                pattern=[[1, P]], channel_multiplier=-1,
            )

            def emit_prep(h, nsplit=2):
                """Load + PE-transpose (f32) + flatten-cast head h."""
                q_raw = stage.tile([P, ST, D], F32, tag="qraw")
                k_raw = stage.tile([P, ST, D], F32, tag="kraw")
                v_raw = stage.tile([P, ST, D], F32, tag="vraw")
                qT3 = tpool.tile([P, ST, P], BF16, tag="qT3")
                kT3 = tpool.tile([P, ST, P], BF16, tag="kT3")
                if h < 2:  # pool slots keep their zero top halves across heads
                    nc.gpsimd.memset(qT3[64:P, :, :], 0.0)
                    nc.gpsimd.memset(kT3[64:P, :, :], 0.0)
                splits = [(ST * i // nsplit, ST * (i + 1) // nsplit)
                          for i in range(nsplit)]
                for s0, s1 in splits:
                    for (raw, d_) in ((q_raw, q_d), (k_raw, k_d)):
                        nc.sync.dma_start(
                            out=raw[:, s0:s1, :],
                            in_=d_[h].rearrange("(b p) d -> p b d", p=P)[:, s0:s1, :])
                for si, (s0, s1) in enumerate(splits):
                    if si == 1 or nsplit == 1:
                        # defer V: only Q/K gate the QK^T ramp
                        nc.sync.dma_start(
                            out=v_raw, in_=v_d[h].rearrange("(b p) d -> p b d", p=P))
                    p0, p1 = s0 // 2, s1 // 2
                    npr = p1 - p0
                    tps = []
                    for (raw, t3, otag) in ((q_raw, qT3, "qodd"),
                                            (k_raw, kT3, "kodd")):
                        # borrow an et rotation slot (PSUM is fully booked:
                        # et 2x3 banks + o4 2x1); transposes run between
                        # pieces at head boundaries
                        tp = ps.tile([P, 4, P], F32, tag="et", bufs=2,
                                     name="tp")
                        tps.append(tp)
                        for i in range(npr):
                            pr = p0 + i
                            nc.tensor.transpose(
                                tp[:, i, :],
                                raw[:, 2 * pr:2 * pr + 2, :].rearrange(
                                    "p a d -> p (a d)"),
                                identity,
                            )
                    for tp, (raw, t3, otag) in zip(
                            tps, ((q_raw, qT3, "qodd"), (k_raw, kT3, "kodd"))):
                        # even s-tiles -> partitions 0:64 of t3 (cast f32->bf16)
                        nc.vector.tensor_copy(t3[0:64, 2 * p0:2 * p1:2, :],
                                              tp[0:64, 0:npr, :])
                        # odd s-tiles: cast to tmp, partition-shift DMA into t3
                        odd = stage.tile([P, 4, P], BF16, tag=otag, name="odd")
                        nc.vector.tensor_copy(odd[64:P, 0:npr, :],
                                              tp[64:P, 0:npr, :])
                        nc.sync.dma_start(
                            out=t3[0:64, 2 * p0 + 1:2 * p1:2, :],
                            in_=odd[64:P, 0:npr, :])
                v_aug = stage.tile([P, ST, D + 1], BF16, tag="vaug")
                nc.vector.tensor_copy(v_aug[:, :, 0:D], v_raw)
                nc.vector.memset(v_aug[:, :, D:D + 1], 1.0)
                at_all = atp.tile([P, AT_TOT], BF16, tag="at")
                return {
                    "qT": qT3.rearrange("p t c -> p (t c)"),
                    "kT": kT3.rearrange("p t c -> p (t c)"),
                    "v_aug": v_aug, "at": at_all,
                    "o_sb": osbp.tile([P, ST, D], F32, tag="osb",
                                      name=f"osb{h}"),
                }

            state = {}

            def emit_piece(h, pi):
                """QK^T matmuls for piece pi of head h, then exp, then masks."""
                st = state[h]
                gpi = h * NPC + pi
                at0, cols, frags, done, diag = PIECES[pi]
                et = ps.tile([P, PIECE], F32, tag="et", bufs=2, name="et")
                for (j, qa, qb) in frags:
                    base = AT_OFF[j] + (qa - P * j) - at0  # piece-local col
                    c, end = base, base + (qb - qa)
                    while c < end:
                        # chunk <=512, not crossing a psum bank boundary
                        w = min(512 - c % 512, end - c)
                        qg = qa + (c - base)
                        nc.tensor.matmul(
                            et[:, c:c + w],
                            lhsT=st["kT"][:, P * j:P * (j + 1)],
                            rhs=st["qT"][:, qg:qg + w],
                            start=True, stop=True,
                        )
                        c += w
                dst = st["at"][:, at0:at0 + cols]
                eng = PAT[gpi % len(PAT)]
                if eng == "A":
                    nc.scalar.activation(
                        dst, et[:, 0:cols],
                        mybir.ActivationFunctionType.Exp,
                        scale=SCALE,
                    )
                elif eng == "D":
                    nc.vector.tensor_scalar(
                        out=dst.bitcast(I16),
                        in0=et[:, 0:cols],
                        scalar1=ALPHA,
                        scalar2=BETA,
                        op0=mybir.AluOpType.mult,
                        op1=mybir.AluOpType.add,
                    )
                else:
                    raise ValueError(f"bad engine {eng}")
                for j in diag:
                    m_ = nc.gpsimd if MASKENG == "gp" else nc.vector
                    dg = st["at"][:, AT_OFF[j]:AT_OFF[j] + P]
                    m_.tensor_tensor(out=dg, in0=dg, in1=trimask,
                                     op=mybir.AluOpType.mult)

            def emit_av(h, jq):
                """A@V for q-tile jq of head h; groups of 4 share a PSUM bank."""
                st = state[h]
                at_all, v_aug, o_sb = st["at"], st["v_aug"], st["o_sb"]
                if jq % 4 == 0:
                    st["o4"] = ps.tile([P, 4, D + 1], F32, tag="o",
                                       bufs=2, name="o4")
                o4 = st["o4"]
                for k in range(jq + 1):
                    c = AT_OFF[k] + P * (jq - k)
                    nc.tensor.matmul(
                        o4[:, jq % 4, :],
                        lhsT=at_all[:, c:c + P],
                        rhs=v_aug[:, k, :],
                        start=(k == 0), stop=(k == jq),
                    )
                if jq % 4 == 3:
                    recip4 = small.tile([P, 4], F32, tag="recip")
                    nc.vector.reciprocal(
                        recip4,
                        o4[:, :, D:D + 1].rearrange("p a b -> p (a b)"),
                    )
                    rb = bass.AP(tensor=recip4.tensor, offset=recip4.offset,
                                 ap=[recip4.ap[0], recip4.ap[1], [0, D]])
                    n_ = nc.vector if NORMENG == "dve" else nc.gpsimd
                    n_.tensor_tensor(
                        out=o_sb[:, jq - 3:jq + 1, :],
                        in0=o4[:, :, 0:D], in1=rb,
                        op=mybir.AluOpType.mult,
                    )
                    nc.sync.dma_start(
                        out=o_d[h].rearrange("(b p) d -> p b d", p=P)
                                  [:, jq - 3:jq + 1, :],
                        in_=o_sb[:, jq - 3:jq + 1, :],
                    )
                if jq == ST - 1:
                    del state[h]

            # flattened pipeline over (head, piece); A@V trails by AVLAG pieces
            av_tasks = [(h, jq) for h in range(heads_per_core)
                        for jq in range(ST)]
            av_ready_gpi = {}
            for h in range(heads_per_core):
                for jq in range(ST):
                    av_ready_gpi[(h, jq)] = h * NPC + READY[jq]
            state[0] = emit_prep(0, nsplit=NSPLIT0)
            av_next = 0
            for h in range(heads_per_core):
                for pi in range(NPC):
                    gpi = h * NPC + pi
                    emit_piece(h, pi)
                    if pi == 7 and h + 1 < heads_per_core:
                        state[h + 1] = emit_prep(h + 1)
                    while (av_next < len(av_tasks)
                           and av_ready_gpi[av_tasks[av_next]] + AVLAG <= gpi):
                        emit_av(*av_tasks[av_next])
                        av_next += 1
            while av_next < len(av_tasks):
                emit_av(*av_tasks[av_next])
                av_next += 1

    nc.compile()
    return nc


_NC_CACHE = {}


def _get_nc(heads_per_core=HPC):
    if heads_per_core not in _NC_CACHE:
        _NC_CACHE[heads_per_core] = build_nc(heads_per_core)
    return _NC_CACHE[heads_per_core]


def run_sharded(Q, K, V, heads_per_core=HPC, **run_kwargs):
    """Q, K, V: [HEADS-or-subset, S, D] f32 flattened over (B, H)."""
    nc = _get_nc(heads_per_core)
    n = heads_per_core
    in_maps = [
        {
            "Q": np.ascontiguousarray(Q[i * n:(i + 1) * n]),
            "K": np.ascontiguousarray(K[i * n:(i + 1) * n]),
            "V": np.ascontiguousarray(V[i * n:(i + 1) * n]),
        }
        for i in range(N_CORES)
    ]
    last_err = None
    for attempt in range(3):
        try:
            res = run_bass_kernel_spmd(nc, in_maps,
                                       core_ids=list(range(N_CORES)),
                                       **run_kwargs)
            out = np.concatenate(
                [np.asarray(res.results[i]["out"]) for i in range(N_CORES)],
                axis=0)
            return out, res
        except Exception as e:  # transient NRT_EXEC_UNIT_UNRECOVERABLE etc.
            last_err = e
            import time
            time.sleep(2.0)
    raise last_err


def kernel(Q, K, V, mask=None):
    Q = np.asarray(Q, dtype=np.float32).reshape(HEADS, S, D)
    K = np.asarray(K, dtype=np.float32).reshape(HEADS, S, D)
    V = np.asarray(V, dtype=np.float32).reshape(HEADS, S, D)
    out, _ = run_sharded(Q, K, V)
    return out.reshape(B, H, S, D)


# revision 15
# speedup vs baseline: 1.9063x; 1.3118x over previous
"""Causal multi-head attention (B=4, H=16, S=2048, D=64) on 8 TRN2 NeuronCores.

Sharding: B*H = 64 heads, 8 heads per core (data/head parallel, no comms).

Per-core pipeline (per head), v3 — exp spread over three engines:
  - DMA Q,K,V [2048,64] f32 -> SBUF
  - Q,K to d-major via PE pair-transposes reading f32 raws directly
    ([128 s, 2x64 d] -> [128,128] f32 PSUM; no separate bf16 cast pass),
    flattened to [128, 2048] bf16 with top 64 partitions zeroed (K=128
    contraction pad): even s-tiles via DVE copy-cast, odd s-tiles via DVE
    copy-cast + partition-shift DMA
  - QK^T strips E^T[k, q] packed into [128, <=1536] f32 PSUM pieces
    (12/head), matmul chunks <=512 cols split at PSUM bank boundaries
  - exp: each piece goes to ONE of: ScalarE ACT Exp, DVE Schraudolph
    (bits = int16(e*ALPHA + BETA) bitcast to bf16), or GpSimd Schraudolph
    after a PSUM->SBUF DMA bounce -- balancing the three engines; strips
    land contiguously in one at_all [128, 17408] bf16 tile per head
  - causal zeroing of diagonal tiles via trimask multiply (gpsimd/DVE)
  - A@V with ones-column appended to V: o4 groups of 4 q-tiles in PSUM,
    normalize with DVE reciprocal + scale, stream out per group
"""

import os
import sys

try:
    import concourse.bass as bass  # noqa: F401
except ImportError:
    sys.path.insert(0, "/opt/trn_rl_repo")
    import concourse.bass as bass  # noqa: F401

import numpy as np

import concourse.mybir as mybir
import concourse.tile as tile
from concourse import bacc
from concourse.bass_utils import run_bass_kernel_spmd
from concourse.masks import make_identity

B, H, S, D = 4, 16, 2048, 64
N_CORES = 8
HEADS = B * H
HPC = HEADS // N_CORES  # heads per core
P = 128
ST = S // P  # 16 s-tiles per head

F32 = mybir.dt.float32
BF16 = mybir.dt.bfloat16
I16 = mybir.dt.int16

SCALE = 1.0 / float(np.sqrt(D))
LN2 = float(np.log(2.0))
ALPHA = 128.0 / LN2 * SCALE          # Schraudolph slope on raw scores
BETA = 127.0 * 128.0 - 7.33          # exponent bias, centered for min RMS err

PIECE = int(os.environ.get("K_PIECE", "1024"))   # PSUM piece columns
AVLAG = int(os.environ.get("K_AVLAG", "1"))      # A@V lag in pieces
# per-piece exp engine pattern (A=ACT, D=DVE, G=GpSimd via SBUF bounce)
PAT = os.environ.get("K_PAT", "ADAADADAADAADAAAA")
MASKENG = os.environ.get("K_MASKENG", "gp")      # gp | dve
NORMENG = os.environ.get("K_NORMENG", "dve")     # dve | gp
NSPLIT0 = int(os.environ.get("K_NSPLIT0", "2"))

WJ = [S - P * j for j in range(ST)]              # strip widths
AT_OFF = [0] * (ST + 1)
for j in range(ST):
    AT_OFF[j + 1] = AT_OFF[j] + WJ[j]
AT_TOT = AT_OFF[ST]                              # 17408


def plan_pieces():
    """Greedy pack strips (in order) into PIECE-col pieces.
    Each piece: (at0, cols, frags, done, diag), frags = [(j, qa, qb)] in
    absolute q coords (128-aligned)."""
    pieces = []
    at0 = 0
    j, q = 0, 0
    while j < ST:
        cols = 0
        frags = []
        done = []
        diag = []
        while j < ST and cols < PIECE:
            take = min(WJ[j] - q, PIECE - cols)
            if q == 0:
                diag.append(j)
            frags.append((j, P * j + q, P * j + q + take))
            q += take
            cols += take
            if q == WJ[j]:
                done.append(j)
                j += 1
                q = 0
        pieces.append((at0, cols, frags, done, diag))
        at0 += cols
    return pieces


PIECES = plan_pieces()
NPC = len(PIECES)
READY = [0] * ST
for pi, (_, _, _, done, _) in enumerate(PIECES):
    for j in done:
        READY[j] = pi


def build_nc(heads_per_core=HPC):
    nc = bacc.Bacc("TRN2", target_bir_lowering=False, debug=False,
                   num_devices=N_CORES)
    q_d = nc.dram_tensor("Q", [heads_per_core, S, D], F32, kind="ExternalInput")
    k_d = nc.dram_tensor("K", [heads_per_core, S, D], F32, kind="ExternalInput")
    v_d = nc.dram_tensor("V", [heads_per_core, S, D], F32, kind="ExternalInput")
    o_d = nc.dram_tensor("out", [heads_per_core, S, D], F32, kind="ExternalOutput")

    with tile.TileContext(nc) as tc:
        with (
            tc.tile_pool(name="const", bufs=1) as const,
            tc.tile_pool(name="stage", bufs=2) as stage,
            tc.tile_pool(name="tp", bufs=2) as tpool,
            tc.tile_pool(name="atp", bufs=2) as atp,
            tc.tile_pool(name="osb", bufs=2) as osbp,
            tc.tile_pool(name="small", bufs=8) as small,
            tc.tile_pool(name="ps", bufs=1, space="PSUM") as ps,
        ):
            identity = const.tile([P, P], BF16, tag="ident")
            make_identity(nc, identity)
            # upper-triangular (incl. diagonal) ones: keep q >= k
            trimask = const.tile([P, P], BF16, tag="trimask")
            nc.gpsimd.memset(trimask, 1.0)
            nc.gpsimd.affine_select(
                out=trimask, in_=trimask,
                compare_op=mybir.AluOpType.is_ge,
                fill=0.0, base=0,
                pattern=[[1, P]], channel_multiplier=-1,
            )

            def emit_prep(h, nsplit=2):
                """Load + cast + PE-transpose (bf16) + flatten head h."""
                q_raw = stage.tile([P, ST, D], F32, tag="qraw")
                k_raw = stage.tile([P, ST, D], F32, tag="kraw")
                v_raw = stage.tile([P, ST, D], F32, tag="vraw")
                q_bf = stage.tile([P, ST, D], BF16, tag="qbf")
                k_bf = stage.tile([P, ST, D], BF16, tag="kbf")
                qT3 = tpool.tile([P, ST, P], BF16, tag="qT3")
                kT3 = tpool.tile([P, ST, P], BF16, tag="kT3")
                if h < 2:  # pool slots keep their zero top halves across heads
                    nc.gpsimd.memset(qT3[64:P, :, :], 0.0)
                    nc.gpsimd.memset(kT3[64:P, :, :], 0.0)
                splits = [(ST * i // nsplit, ST * (i + 1) // nsplit)
                          for i in range(nsplit)]
                for s0, s1 in splits:
                    for (raw, d_) in ((q_raw, q_d), (k_raw, k_d)):
                        nc.sync.dma_start(
                            out=raw[:, s0:s1, :],
                            in_=d_[h].rearrange("(b p) d -> p b d", p=P)[:, s0:s1, :])
                for si, (s0, s1) in enumerate(splits):
                    if si == 1 or nsplit == 1:
                        # defer V: only Q/K gate the QK^T ramp
                        nc.sync.dma_start(
                            out=v_raw, in_=v_d[h].rearrange("(b p) d -> p b d", p=P))
                    p0, p1 = s0 // 2, s1 // 2
                    npr = p1 - p0
                    # phase 1: casts (DVE) for both operands
                    for (raw, bf_) in ((q_raw, q_bf), (k_raw, k_bf)):
                        nc.vector.tensor_copy(bf_[:, s0:s1, :], raw[:, s0:s1, :])
                    # phase 2: PE pair-transposes (bf16) into borrowed et slots
                    tps = []
                    for (bf_, t3, otag) in ((q_bf, qT3, "qodd"),
                                            (k_bf, kT3, "kodd")):
                        # borrow an et rotation slot (PSUM is fully booked:
                        # et 3x2 banks + o4 2x1); transposes run between
                        # pieces at head boundaries
                        tp = ps.tile([P, 4, P], BF16, tag="et", bufs=3,
                                     name="tp")
                        tps.append(tp)
                        for i in range(npr):
                            pr = p0 + i
                            nc.tensor.transpose(
                                tp[:, i, :],
                                bf_[:, 2 * pr:2 * pr + 2, :].rearrange(
                                    "p a d -> p (a d)"),
                                identity,
                            )
                    # phase 3: flatten (DVE) + odd partition-shift DMA
                    for tp, (bf_, t3, otag) in zip(
                            tps, ((q_bf, qT3, "qodd"), (k_bf, kT3, "kodd"))):
                        nc.vector.tensor_copy(t3[0:64, 2 * p0:2 * p1:2, :],
                                              tp[0:64, 0:npr, :])
                        odd = stage.tile([P, 4, P], BF16, tag=otag, name="odd")
                        nc.vector.tensor_copy(odd[64:P, 0:npr, :],
                                              tp[64:P, 0:npr, :])
                        nc.sync.dma_start(
                            out=t3[0:64, 2 * p0 + 1:2 * p1:2, :],
                            in_=odd[64:P, 0:npr, :])
                v_aug = stage.tile([P, ST, D + 1], BF16, tag="vaug")
                nc.gpsimd.tensor_copy(v_aug[:, :, 0:D], v_raw)
                nc.gpsimd.memset(v_aug[:, :, D:D + 1], 1.0)
                at_all = atp.tile([P, AT_TOT], BF16, tag="at")
                return {
                    "qT": qT3.rearrange("p t c -> p (t c)"),
                    "kT": kT3.rearrange("p t c -> p (t c)"),
                    "v_aug": v_aug, "at": at_all,
                    "o_sb": osbp.tile([P, ST, D], F32, tag="osb",
                                      name=f"osb{h}"),
                }

            state = {}

            def emit_piece(h, pi):
                """QK^T matmuls for piece pi of head h, then exp, then masks."""
                st = state[h]
                gpi = h * NPC + pi
                at0, cols, frags, done, diag = PIECES[pi]
                et = ps.tile([P, PIECE], F32, tag="et", bufs=3, name="et")
                for (j, qa, qb) in frags:
                    base = AT_OFF[j] + (qa - P * j) - at0  # piece-local col
                    c, end = base, base + (qb - qa)
                    while c < end:
                        # chunk <=512, not crossing a psum bank boundary
                        w = min(512 - c % 512, end - c)
                        qg = qa + (c - base)
                        nc.tensor.matmul(
                            et[:, c:c + w],
                            lhsT=st["kT"][:, P * j:P * (j + 1)],
                            rhs=st["qT"][:, qg:qg + w],
                            start=True, stop=True,
                        )
                        c += w
                dst = st["at"][:, at0:at0 + cols]
                eng = PAT[gpi % len(PAT)]
                if eng == "A":
                    nc.scalar.activation(
                        dst, et[:, 0:cols],
                        mybir.ActivationFunctionType.Exp,
                        scale=SCALE,
                    )
                elif eng == "D":
                    nc.vector.tensor_scalar(
                        out=dst.bitcast(I16),
                        in0=et[:, 0:cols],
                        scalar1=ALPHA,
                        scalar2=BETA,
                        op0=mybir.AluOpType.mult,
                        op1=mybir.AluOpType.add,
                    )
                else:
                    raise ValueError(f"bad engine {eng}")
                for j in diag:
                    m_ = nc.gpsimd if MASKENG == "gp" else nc.vector
                    dg = st["at"][:, AT_OFF[j]:AT_OFF[j] + P]
                    m_.tensor_tensor(out=dg, in0=dg, in1=trimask,
                                     op=mybir.AluOpType.mult)

            def emit_av(h, jq):
                """A@V for q-tile jq of head h; groups of 4 share a PSUM bank."""
                st = state[h]
                at_all, v_aug, o_sb = st["at"], st["v_aug"], st["o_sb"]
                if jq % 4 == 0:
                    st["o4"] = ps.tile([P, 4, D + 1], F32, tag="o",
                                       bufs=2, name="o4")
                o4 = st["o4"]
                for k in range(jq + 1):
                    c = AT_OFF[k] + P * (jq - k)
                    nc.tensor.matmul(
                        o4[:, jq % 4, :],
                        lhsT=at_all[:, c:c + P],
                        rhs=v_aug[:, k, :],
                        start=(k == 0), stop=(k == jq),
                    )
                if jq % 4 == 3:
                    recip4 = small.tile([P, 4], F32, tag="recip")
                    nc.vector.reciprocal(
                        recip4,
                        o4[:, :, D:D + 1].rearrange("p a b -> p (a b)"),
                    )
                    rb = bass.AP(tensor=recip4.tensor, offset=recip4.offset,
                                 ap=[recip4.ap[0], recip4.ap[1], [0, D]])
                    n_ = nc.vector if NORMENG == "dve" else nc.gpsimd
                    n_.tensor_tensor(
                        out=o_sb[:, jq - 3:jq + 1, :],
                        in0=o4[:, :, 0:D], in1=rb,
                        op=mybir.AluOpType.mult,
                    )
                    nc.sync.dma_start(
                        out=o_d[h].rearrange("(b p) d -> p b d", p=P)
                                  [:, jq - 3:jq + 1, :],
                        in_=o_sb[:, jq - 3:jq + 1, :],
                    )
                if jq == ST - 1:
                    del state[h]

            # flattened pipeline over (head, piece); A@V trails by AVLAG pieces
            av_tasks = [(h, jq) for h in range(heads_per_core)
                        for jq in range(ST)]
            av_ready_gpi = {}
            for h in range(heads_per_core):
                for jq in range(ST):
                    av_ready_gpi[(h, jq)] = h * NPC + READY[jq]
            state[0] = emit_prep(0, nsplit=NSPLIT0)
            av_next = 0
            for h in range(heads_per_core):
                for pi in range(NPC):
                    gpi = h * NPC + pi
                    emit_piece(h, pi)
                    if pi == 7 and h + 1 < heads_per_core:
                        state[h + 1] = emit_prep(h + 1)
                    while (av_next < len(av_tasks)
                           and av_ready_gpi[av_tasks[av_next]] + AVLAG <= gpi):
                        emit_av(*av_tasks[av_next])
                        av_next += 1
            while av_next < len(av_tasks):
                emit_av(*av_tasks[av_next])
                av_next += 1

    nc.compile()
    return nc


_NC_CACHE = {}


def _get_nc(heads_per_core=HPC):
    if heads_per_core not in _NC_CACHE:
        _NC_CACHE[heads_per_core] = build_nc(heads_per_core)
    return _NC_CACHE[heads_per_core]


def run_sharded(Q, K, V, heads_per_core=HPC, **run_kwargs):
    """Q, K, V: [HEADS-or-subset, S, D] f32 flattened over (B, H)."""
    nc = _get_nc(heads_per_core)
    n = heads_per_core
    in_maps = [
        {
            "Q": np.ascontiguousarray(Q[i * n:(i + 1) * n]),
            "K": np.ascontiguousarray(K[i * n:(i + 1) * n]),
            "V": np.ascontiguousarray(V[i * n:(i + 1) * n]),
        }
        for i in range(N_CORES)
    ]
    last_err = None
    for attempt in range(3):
        try:
            res = run_bass_kernel_spmd(nc, in_maps,
                                       core_ids=list(range(N_CORES)),
                                       **run_kwargs)
            out = np.concatenate(
                [np.asarray(res.results[i]["out"]) for i in range(N_CORES)],
                axis=0)
            return out, res
        except Exception as e:  # transient NRT_EXEC_UNIT_UNRECOVERABLE etc.
            last_err = e
            import time
            time.sleep(2.0)
    raise last_err


def kernel(Q, K, V, mask=None):
    Q = np.asarray(Q, dtype=np.float32).reshape(HEADS, S, D)
    K = np.asarray(K, dtype=np.float32).reshape(HEADS, S, D)
    V = np.asarray(V, dtype=np.float32).reshape(HEADS, S, D)
    out, _ = run_sharded(Q, K, V)
    return out.reshape(B, H, S, D)


# revision 16
# speedup vs baseline: 2.1272x; 1.1159x over previous
"""Causal multi-head attention (B=4, H=16, S=2048, D=64) on 8 TRN2 NeuronCores.

Sharding: B*H = 64 heads, 8 heads per core (data/head parallel, no comms).

Per-core pipeline (per head), v3 — exp spread over three engines:
  - DMA Q,K,V [2048,64] f32 -> SBUF
  - Q,K to d-major via PE pair-transposes reading f32 raws directly
    ([128 s, 2x64 d] -> [128,128] f32 PSUM; no separate bf16 cast pass),
    flattened to [128, 2048] bf16 with top 64 partitions zeroed (K=128
    contraction pad): even s-tiles via DVE copy-cast, odd s-tiles via DVE
    copy-cast + partition-shift DMA
  - QK^T strips E^T[k, q] packed into [128, <=1536] f32 PSUM pieces
    (12/head), matmul chunks <=512 cols split at PSUM bank boundaries
  - exp: each piece goes to ONE of: ScalarE ACT Exp, DVE Schraudolph
    (bits = int16(e*ALPHA + BETA) bitcast to bf16), or GpSimd Schraudolph
    after a PSUM->SBUF DMA bounce -- balancing the three engines; strips
    land contiguously in one at_all [128, 17408] bf16 tile per head
  - causal zeroing of diagonal tiles via trimask multiply (gpsimd/DVE)
  - A@V with ones-column appended to V: o4 groups of 4 q-tiles in PSUM,
    normalize with DVE reciprocal + scale, stream out per group
"""

import os
import sys

try:
    import concourse.bass as bass  # noqa: F401
except ImportError:
    sys.path.insert(0, "/opt/trn_rl_repo")
    import concourse.bass as bass  # noqa: F401

import numpy as np

import concourse.mybir as mybir
import concourse.tile as tile
from concourse import bacc
from concourse.bass_utils import run_bass_kernel_spmd
from concourse.masks import make_identity

B, H, S, D = 4, 16, 2048, 64
N_CORES = 8
HEADS = B * H
HPC = HEADS // N_CORES  # heads per core
P = 128
ST = S // P  # 16 s-tiles per head

F32 = mybir.dt.float32
BF16 = mybir.dt.bfloat16
I16 = mybir.dt.int16

SCALE = 1.0 / float(np.sqrt(D))
LN2 = float(np.log(2.0))
ALPHA = 128.0 / LN2 * SCALE          # Schraudolph slope on raw scores
BETA = 127.0 * 128.0 - 7.33          # exponent bias, centered for min RMS err

PIECE = int(os.environ.get("K_PIECE", "1024"))   # PSUM piece columns
AVLAG = int(os.environ.get("K_AVLAG", "0"))      # A@V lag in pieces
# per-piece exp engine pattern (A=ACT, D=DVE, G=GpSimd via SBUF bounce)
PAT = os.environ.get("K_PAT", "ADAADADAADAADAAAA")
MASKENG = os.environ.get("K_MASKENG", "gp")      # gp | dve
NORMENG = os.environ.get("K_NORMENG", "dve")     # dve | gp
NSPLIT0 = int(os.environ.get("K_NSPLIT0", "2"))

WJ = [S - P * j for j in range(ST)]              # strip widths
AT_OFF = [0] * (ST + 1)
for j in range(ST):
    AT_OFF[j + 1] = AT_OFF[j] + WJ[j]
AT_TOT = AT_OFF[ST]                              # 17408


def plan_pieces():
    """Greedy pack strips (in order) into PIECE-col pieces.
    Each piece: (at0, cols, frags, done, diag), frags = [(j, qa, qb)] in
    absolute q coords (128-aligned)."""
    pieces = []
    at0 = 0
    j, q = 0, 0
    while j < ST:
        cols = 0
        frags = []
        done = []
        diag = []
        while j < ST and cols < PIECE:
            take = min(WJ[j] - q, PIECE - cols)
            if q == 0:
                diag.append(j)
            frags.append((j, P * j + q, P * j + q + take))
            q += take
            cols += take
            if q == WJ[j]:
                done.append(j)
                j += 1
                q = 0
        pieces.append((at0, cols, frags, done, diag))
        at0 += cols
    return pieces


PIECES = plan_pieces()
NPC = len(PIECES)
READY = [0] * ST
for pi, (_, _, _, done, _) in enumerate(PIECES):
    for j in done:
        READY[j] = pi


def build_nc(heads_per_core=HPC):
    nc = bacc.Bacc("TRN2", target_bir_lowering=False, debug=False,
                   num_devices=N_CORES)
    q_d = nc.dram_tensor("Q", [heads_per_core, S, D], F32, kind="ExternalInput")
    k_d = nc.dram_tensor("K", [heads_per_core, S, D], F32, kind="ExternalInput")
    v_d = nc.dram_tensor("V", [heads_per_core, S, D], F32, kind="ExternalInput")
    o_d = nc.dram_tensor("out", [heads_per_core, S, D], F32, kind="ExternalOutput")

    with tile.TileContext(nc) as tc:
        with (
            tc.tile_pool(name="const", bufs=1) as const,
            tc.tile_pool(name="stage", bufs=2) as stage,
            tc.tile_pool(name="tp", bufs=2) as tpool,
            tc.tile_pool(name="atp", bufs=2) as atp,
            tc.tile_pool(name="osb", bufs=2) as osbp,
            tc.tile_pool(name="small", bufs=8) as small,
            tc.tile_pool(name="ps", bufs=1, space="PSUM") as ps,
        ):
            identity = const.tile([P, P], BF16, tag="ident")
            make_identity(nc, identity)
            # upper-triangular (incl. diagonal) ones: keep q >= k
            trimask = const.tile([P, P], BF16, tag="trimask")
            nc.gpsimd.memset(trimask, 1.0)
            nc.gpsimd.affine_select(
                out=trimask, in_=trimask,
                compare_op=mybir.AluOpType.is_ge,
                fill=0.0, base=0,
                pattern=[[1, P]], channel_multiplier=-1,
            )

            def emit_prep(h, nsplit=2):
                """Load + cast + PE-transpose (bf16) + flatten head h."""
                q_raw = stage.tile([P, ST, D], F32, tag="qraw")
                k_raw = stage.tile([P, ST, D], F32, tag="kraw")
                v_raw = stage.tile([P, ST, D], F32, tag="vraw")
                q_bf = stage.tile([P, ST, D], BF16, tag="qbf")
                k_bf = stage.tile([P, ST, D], BF16, tag="kbf")
                qT3 = tpool.tile([P, ST, P], BF16, tag="qT3")
                kT3 = tpool.tile([P, ST, P], BF16, tag="kT3")
                if h < 2:  # pool slots keep their zero top halves across heads
                    nc.vector.memset(qT3[64:P, :, :], 0.0)
                    nc.vector.memset(kT3[64:P, :, :], 0.0)
                splits = [(ST * i // nsplit, ST * (i + 1) // nsplit)
                          for i in range(nsplit)]
                for s0, s1 in splits:
                    for (raw, d_) in ((q_raw, q_d), (k_raw, k_d)):
                        nc.sync.dma_start(
                            out=raw[:, s0:s1, :],
                            in_=d_[h].rearrange("(b p) d -> p b d", p=P)[:, s0:s1, :])
                for si, (s0, s1) in enumerate(splits):
                    if si == 1 or nsplit == 1:
                        # defer V: only Q/K gate the QK^T ramp
                        nc.sync.dma_start(
                            out=v_raw, in_=v_d[h].rearrange("(b p) d -> p b d", p=P))
                    p0, p1 = s0 // 2, s1 // 2
                    npr = p1 - p0
                    # phase 1: casts (DVE) for both operands
                    for (raw, bf_) in ((q_raw, q_bf), (k_raw, k_bf)):
                        nc.vector.tensor_copy(bf_[:, s0:s1, :], raw[:, s0:s1, :])
                    # phase 2: PE pair-transposes (bf16) into borrowed et slots
                    tps = []
                    for (bf_, t3, otag) in ((q_bf, qT3, "qodd"),
                                            (k_bf, kT3, "kodd")):
                        # borrow an et rotation slot (PSUM is fully booked:
                        # et 3x2 banks + o4 2x1); transposes run between
                        # pieces at head boundaries
                        tp = ps.tile([P, 4, P], BF16, tag="et", bufs=3,
                                     name="tp")
                        tps.append(tp)
                        for i in range(npr):
                            pr = p0 + i
                            nc.tensor.transpose(
                                tp[:, i, :],
                                bf_[:, 2 * pr:2 * pr + 2, :].rearrange(
                                    "p a d -> p (a d)"),
                                identity,
                            )
                    # phase 3: flatten (DVE) + odd partition-shift DMA
                    for tp, (bf_, t3, otag) in zip(
                            tps, ((q_bf, qT3, "qodd"), (k_bf, kT3, "kodd"))):
                        nc.vector.tensor_copy(t3[0:64, 2 * p0:2 * p1:2, :],
                                              tp[0:64, 0:npr, :])
                        odd = stage.tile([P, 4, P], BF16, tag=otag, name="odd")
                        nc.vector.tensor_copy(odd[64:P, 0:npr, :],
                                              tp[64:P, 0:npr, :])
                        nc.sync.dma_start(
                            out=t3[0:64, 2 * p0 + 1:2 * p1:2, :],
                            in_=odd[64:P, 0:npr, :])
                v_aug = stage.tile([P, ST, D + 1], BF16, tag="vaug")
                nc.vector.tensor_copy(v_aug[:, :, 0:D], v_raw)
                nc.vector.memset(v_aug[:, :, D:D + 1], 1.0)
                at_all = atp.tile([P, AT_TOT], BF16, tag="at")
                return {
                    "qT": qT3.rearrange("p t c -> p (t c)"),
                    "kT": kT3.rearrange("p t c -> p (t c)"),
                    "v_aug": v_aug, "at": at_all,
                    "o_sb": osbp.tile([P, ST, D], F32, tag="osb",
                                      name=f"osb{h}"),
                }

            state = {}

            def emit_piece(h, pi):
                """QK^T matmuls for piece pi of head h, then exp, then masks."""
                st = state[h]
                gpi = h * NPC + pi
                at0, cols, frags, done, diag = PIECES[pi]
                et = ps.tile([P, PIECE], F32, tag="et", bufs=3, name="et")
                for (j, qa, qb) in frags:
                    base = AT_OFF[j] + (qa - P * j) - at0  # piece-local col
                    c, end = base, base + (qb - qa)
                    while c < end:
                        # chunk <=512, not crossing a psum bank boundary
                        w = min(512 - c % 512, end - c)
                        qg = qa + (c - base)
                        nc.tensor.matmul(
                            et[:, c:c + w],
                            lhsT=st["kT"][:, P * j:P * (j + 1)],
                            rhs=st["qT"][:, qg:qg + w],
                            start=True, stop=True,
                        )
                        c += w
                dst = st["at"][:, at0:at0 + cols]
                eng = PAT[gpi % len(PAT)]
                if eng == "A":
                    nc.scalar.activation(
                        dst, et[:, 0:cols],
                        mybir.ActivationFunctionType.Exp,
                        scale=SCALE,
                    )
                elif eng == "D":
                    nc.vector.tensor_scalar(
                        out=dst.bitcast(I16),
                        in0=et[:, 0:cols],
                        scalar1=ALPHA,
                        scalar2=BETA,
                        op0=mybir.AluOpType.mult,
                        op1=mybir.AluOpType.add,
                    )
                else:
                    raise ValueError(f"bad engine {eng}")
                for j in diag:
                    m_ = nc.gpsimd if MASKENG == "gp" else nc.vector
                    dg = st["at"][:, AT_OFF[j]:AT_OFF[j] + P]
                    m_.tensor_tensor(out=dg, in0=dg, in1=trimask,
                                     op=mybir.AluOpType.mult)

            def emit_av(h, jq):
                """A@V for q-tile jq of head h; groups of 4 share a PSUM bank."""
                st = state[h]
                at_all, v_aug, o_sb = st["at"], st["v_aug"], st["o_sb"]
                if jq % 4 == 0:
                    st["o4"] = ps.tile([P, 4, D + 1], F32, tag="o",
                                       bufs=2, name="o4")
                o4 = st["o4"]
                for k in range(jq + 1):
                    c = AT_OFF[k] + P * (jq - k)
                    nc.tensor.matmul(
                        o4[:, jq % 4, :],
                        lhsT=at_all[:, c:c + P],
                        rhs=v_aug[:, k, :],
                        start=(k == 0), stop=(k == jq),
                    )
                if jq % 4 == 3:
                    recip4 = small.tile([P, 4], F32, tag="recip")
                    nc.vector.reciprocal(
                        recip4,
                        o4[:, :, D:D + 1].rearrange("p a b -> p (a b)"),
                    )
                    rb = bass.AP(tensor=recip4.tensor, offset=recip4.offset,
                                 ap=[recip4.ap[0], recip4.ap[1], [0, D]])
                    n_ = nc.vector if NORMENG == "dve" else nc.gpsimd
                    n_.tensor_tensor(
                        out=o_sb[:, jq - 3:jq + 1, :],
                        in0=o4[:, :, 0:D], in1=rb,
                        op=mybir.AluOpType.mult,
                    )
                    nc.sync.dma_start(
                        out=o_d[h].rearrange("(b p) d -> p b d", p=P)
                                  [:, jq - 3:jq + 1, :],
                        in_=o_sb[:, jq - 3:jq + 1, :],
                    )
                if jq == ST - 1:
                    del state[h]

            # flattened pipeline over (head, piece); A@V trails by AVLAG pieces
            av_tasks = [(h, jq) for h in range(heads_per_core)
                        for jq in range(ST)]
            av_ready_gpi = {}
            for h in range(heads_per_core):
                for jq in range(ST):
                    av_ready_gpi[(h, jq)] = h * NPC + READY[jq]
            state[0] = emit_prep(0, nsplit=NSPLIT0)
            av_next = 0
            for h in range(heads_per_core):
                for pi in range(NPC):
                    gpi = h * NPC + pi
                    emit_piece(h, pi)
                    if pi == 7 and h + 1 < heads_per_core:
                        state[h + 1] = emit_prep(h + 1)
                    while (av_next < len(av_tasks)
                           and av_ready_gpi[av_tasks[av_next]] + AVLAG <= gpi):
                        emit_av(*av_tasks[av_next])
                        av_next += 1
            while av_next < len(av_tasks):
                emit_av(*av_tasks[av_next])
                av_next += 1

    nc.compile()
    return nc


_NC_CACHE = {}


def _get_nc(heads_per_core=HPC):
    if heads_per_core not in _NC_CACHE:
        _NC_CACHE[heads_per_core] = build_nc(heads_per_core)
    return _NC_CACHE[heads_per_core]


def run_sharded(Q, K, V, heads_per_core=HPC, **run_kwargs):
    """Q, K, V: [HEADS-or-subset, S, D] f32 flattened over (B, H)."""
    nc = _get_nc(heads_per_core)
    n = heads_per_core
    in_maps = [
        {
            "Q": np.ascontiguousarray(Q[i * n:(i + 1) * n]),
            "K": np.ascontiguousarray(K[i * n:(i + 1) * n]),
            "V": np.ascontiguousarray(V[i * n:(i + 1) * n]),
        }
        for i in range(N_CORES)
    ]
    last_err = None
    for attempt in range(3):
        try:
            res = run_bass_kernel_spmd(nc, in_maps,
                                       core_ids=list(range(N_CORES)),
                                       **run_kwargs)
            out = np.concatenate(
                [np.asarray(res.results[i]["out"]) for i in range(N_CORES)],
                axis=0)
            return out, res
        except Exception as e:  # transient NRT_EXEC_UNIT_UNRECOVERABLE etc.
            last_err = e
            import time
            time.sleep(2.0)
    raise last_err


def kernel(Q, K, V, mask=None):
    Q = np.asarray(Q, dtype=np.float32).reshape(HEADS, S, D)
    K = np.asarray(K, dtype=np.float32).reshape(HEADS, S, D)
    V = np.asarray(V, dtype=np.float32).reshape(HEADS, S, D)
    out, _ = run_sharded(Q, K, V)
    return out.reshape(B, H, S, D)
